# revision 1
# baseline (speedup 1.0000x reference)
"""CrossAttentionFusion Trainium2 kernel (nn_CrossAttentionFusion__45561013076033).

Full inputs -> full output. Sharding: 8 cores, core c handles batch b=c//2,
query-half h=c%2 (2048 of 4096 queries). Each core holds the full [256,4096]
cnn feature map of its batch (keys/values), its query-half of the transformer
features, and replicated weights.

Per-core dataflow (channel-major [C, N] layouts throughout):
  Q = (Wq X_trf + bq)/16          [256, 2048]  fp32r matmuls
  K = Wk X_cnn + bk               [256, 4096]
  V^T = X_cnn^T Wv^T              [4096, 256]  (bv folded into conv bias)
  per 128-query block:
    S = Q_blk^T K                 [128, 4096]  fp32r
    P = exp(S) (no max-sub; scores are O(1)), row sums via ACT accum_out
    PT = P^T diag(256/rowsum)     via fp16 matmul with scaled identity
  per 256-query superblock:
    A' = V^T^T PT = 256 * attended_norm   [256, 256]  fp16 matmuls, fp32 acc
    out = Wf1 X_trf + (Wf2/256) A' + (bf + Wf2 bv)    fp32r
"""

import numpy as np

B, C, H, W = 4, 256, 64, 64
N = H * W            # 4096 tokens
NCORES = 8
QH = N // 2          # 2048 queries per core
CT = C // 128        # 2 channel tiles
KC = N // 512        # 8 key chunks of 512
NQB = QH // 128      # 16 query blocks per core
NSB = QH // 256      # 8 superblocks per core
NKT = N // 128       # 32 key tiles

_CACHE = {}


def _build():
    import concourse.bass as bass
    import concourse.mybir as mybir
    import concourse.tile as tile
    from concourse import bacc
    from concourse.masks import make_identity

    f32 = mybir.dt.float32
    f32r = mybir.dt.float32r
    f16 = mybir.dt.float16
    AF = mybir.ActivationFunctionType

    nc = bacc.Bacc("TRN2", target_bir_lowering=False, debug=True)

    XQ = nc.dram_tensor("xq", [C, QH], f32, kind="ExternalInput")
    XC = nc.dram_tensor("xc", [C, N], f32, kind="ExternalInput")
    WQT = nc.dram_tensor("wqt", [C, C], f32, kind="ExternalInput")
    WKT = nc.dram_tensor("wkt", [C, C], f32, kind="ExternalInput")
    WVT = nc.dram_tensor("wvt", [C, C], f32, kind="ExternalInput")
    WFT = nc.dram_tensor("wft", [2 * C, C], f32, kind="ExternalInput")
    BQ = nc.dram_tensor("bq", [C], f32, kind="ExternalInput")
    BK = nc.dram_tensor("bk", [C], f32, kind="ExternalInput")
    BF = nc.dram_tensor("bf", [C], f32, kind="ExternalInput")
    OUT = nc.dram_tensor("out", [C, QH], f32, kind="ExternalOutput")

    xq_d = XQ.ap().bitcast(f32r).rearrange("(t p) n -> p t n", p=128)
    xc_d = XC.ap().bitcast(f32r).rearrange("(t p) n -> p t n", p=128)
    wq_d = WQT.ap().bitcast(f32r).rearrange("(t p) d -> p t d", p=128)
    wk_d = WKT.ap().bitcast(f32r).rearrange("(t p) d -> p t d", p=128)
    wv_d = WVT.ap().bitcast(f32r).rearrange("(t p) d -> p t d", p=128)
    wf_d = WFT.ap().bitcast(f32r).rearrange("(t p) d -> p t d", p=128)
    out_d = OUT.ap().rearrange("(t p) n -> p t n", p=128)

    with tile.TileContext(nc) as tc:
        with tc.tile_pool(name="persist", bufs=1) as per, \
             tc.tile_pool(name="soft", bufs=2) as soft, \
             tc.tile_pool(name="ptp", bufs=2) as ptp, \
             tc.tile_pool(name="outp", bufs=2) as outp, \
             tc.tile_pool(name="mm", bufs=2, space="PSUM") as mmp, \
             tc.tile_pool(name="tp", bufs=2, space="PSUM") as tpp, \
             tc.tile_pool(name="av", bufs=2, space="PSUM") as avp:

            # ---- persistent tiles ----
            xq_sb = per.tile([128, CT, QH], f32r)
            xc_sb = per.tile([128, CT, N], f32r)
            wq_sb = per.tile([128, CT, C], f32r)
            wk_sb = per.tile([128, CT, C], f32r)
            wv_sb = per.tile([128, CT, C], f32r)
            wf_sb = per.tile([128, 2 * CT, C], f32r)
            bq_sb = per.tile([128, CT], f32)
            bk_sb = per.tile([128, CT], f32)
            bf_sb = per.tile([128, CT], f32)
            q_sb = per.tile([128, CT, QH], f32r)
            k_sb = per.tile([128, CT, N], f32r)
            vt_sb = per.tile([128, NKT, C], f16)
            ident = per.tile([128, 128], f16)

            nc.sync.dma_start(wq_sb[:], wq_d)
            nc.sync.dma_start(wk_sb[:], wk_d)
            nc.sync.dma_start(wv_sb[:], wv_d)
            nc.sync.dma_start(wf_sb[:], wf_d)
            nc.sync.dma_start(bq_sb[:], BQ.ap().rearrange("(t p) -> p t", p=128))
            nc.sync.dma_start(bk_sb[:], BK.ap().rearrange("(t p) -> p t", p=128))
            nc.sync.dma_start(bf_sb[:], BF.ap().rearrange("(t p) -> p t", p=128))
            make_identity(nc, ident[:])
            for ct in range(CT):
                nc.sync.dma_start(xq_sb[:, ct], xq_d[:, ct])
            for ct in range(CT):
                nc.sync.dma_start(xc_sb[:, ct], xc_d[:, ct])

            # ---- Q projection: Q[d, n] (scaled by 1/16 via host weights) ----
            for dt in range(CT):
                for qc in range(QH // 512):
                    ps = mmp.tile([128, 512], f32, tag="mm512")
                    for ct in range(CT):
                        nc.tensor.matmul(
                            ps[:], wq_sb[:, ct, dt * 128:(dt + 1) * 128],
                            xq_sb[:, ct, qc * 512:(qc + 1) * 512],
                            start=(ct == 0), stop=(ct == CT - 1))
                    nc.scalar.activation(
                        q_sb[:, dt, qc * 512:(qc + 1) * 512], ps[:],
                        AF.Identity, bias=bq_sb[:, dt:dt + 1])

            # ---- K projection ----
            for dt in range(CT):
                for kc in range(KC):
                    ps = mmp.tile([128, 512], f32, tag="mm512")
                    for ct in range(CT):
                        nc.tensor.matmul(
                            ps[:], wk_sb[:, ct, dt * 128:(dt + 1) * 128],
                            xc_sb[:, ct, kc * 512:(kc + 1) * 512],
                            start=(ct == 0), stop=(ct == CT - 1))
                    nc.scalar.activation(
                        k_sb[:, dt, kc * 512:(kc + 1) * 512], ps[:],
                        AF.Identity, bias=bk_sb[:, dt:dt + 1])

            # ---- V^T: [keys, d] (no bias; folded into conv bias) ----
            for mt in range(NKT):
                ps = mmp.tile([128, 512], f32, tag="mm512")
                for ct in range(CT):
                    nc.tensor.matmul(
                        ps[:, :C], xc_sb[:, ct, mt * 128:(mt + 1) * 128],
                        wv_sb[:, ct],
                        start=(ct == 0), stop=(ct == CT - 1))
                nc.scalar.activation(vt_sb[:, mt], ps[:, :C], AF.Copy)

            # ---- attention + fused conv, per 256-query superblock ----
            for sb in range(NSB):
                pt_sb = ptp.tile([128, NKT, 256], f16, tag="pt")
                for qj in range(2):
                    qb = 2 * sb + qj
                    p_sb = soft.tile([128, N], f16, tag="p")
                    sums = soft.tile([128, KC], f32, tag="sums")
                    # S = Q_blk^T K, chunk by 512 keys; exp + row-sum
                    for kc in range(KC):
                        ps = mmp.tile([128, 512], f32, tag="mm512")
                        for ct in range(CT):
                            nc.tensor.matmul(
                                ps[:], q_sb[:, ct, qb * 128:(qb + 1) * 128],
                                k_sb[:, ct, kc * 512:(kc + 1) * 512],
                                start=(ct == 0), stop=(ct == CT - 1))
                        nc.scalar.activation(
                            p_sb[:, kc * 512:(kc + 1) * 512], ps[:],
                            AF.Exp, accum_out=sums[:, kc:kc + 1])
                    ssum = soft.tile([128, 1], f32, tag="ssum")
                    nc.vector.reduce_sum(ssum[:], sums[:],
                                         axis=mybir.AxisListType.X)
                    rinv = soft.tile([128, 1], f32, tag="rinv")
                    nc.vector.reciprocal(rinv[:], ssum[:])
                    r256 = soft.tile([128, 1], f32, tag="r256")
                    nc.vector.tensor_scalar_mul(r256[:], rinv[:], 256.0)
                    sid = soft.tile([128, 128], f16, tag="sid")
                    nc.vector.tensor_scalar_mul(sid[:], ident[:], r256[:])
                    # PT[k, q] = P[q, k] * 256/rowsum[q] via fp16 matmul
                    for g in range(NKT // 4):
                        tps = tpp.tile([128, 4, 128], f32, tag="tp")
                        for j in range(4):
                            kt = 4 * g + j
                            nc.tensor.matmul(
                                tps[:, j], p_sb[:, kt * 128:(kt + 1) * 128],
                                sid[:], start=True, stop=True)
                        nc.vector.tensor_copy(
                            pt_sb[:, 4 * g:4 * (g + 1),
                                  qj * 128:(qj + 1) * 128], tps[:])

                # A' = sum_k VT[k, :]^T PT[k, :]  -> [256 d, 256 q]
                aps = avp.tile([128, CT, 256], f32, tag="av")
                for kt in range(NKT):
                    for dt in range(CT):
                        nc.tensor.matmul(
                            aps[:, dt], vt_sb[:, kt, dt * 128:(dt + 1) * 128],
                            pt_sb[:, kt],
                            start=(kt == 0), stop=(kt == NKT - 1))
                a_sb = outp.tile([128, CT, 256], f32r, tag="a")
                nc.scalar.activation(a_sb[:], aps[:], AF.Copy)

                # fused conv: out = Wf1 xq + Wf2' A' + bf2
                ops = avp.tile([128, CT, 256], f32, tag="conv")
                for dt in range(CT):
                    for kt in range(2 * CT):
                        rhs = (xq_sb[:, kt, sb * 256:(sb + 1) * 256] if kt < CT
                               else a_sb[:, kt - CT])
                        nc.tensor.matmul(
                            ops[:, dt], wf_sb[:, kt, dt * 128:(dt + 1) * 128],
                            rhs, start=(kt == 0), stop=(kt == 2 * CT - 1))
                o_sb = outp.tile([128, CT, 256], f32, tag="o")
                for dt in range(CT):
                    nc.scalar.activation(o_sb[:, dt], ops[:, dt],
                                         AF.Identity, bias=bf_sb[:, dt:dt + 1])
                nc.sync.dma_start(out_d[:, :, sb * 256:(sb + 1) * 256], o_sb[:])

    nc.finalize()
    return nc


def _get_nc():
    if "nc" not in _CACHE:
        _CACHE["nc"] = _build()
    return _CACHE["nc"]


def _in_maps(transformer_features, cnn_features, Wq, bq, Wk, bk, Wv, bv, Wf, bf):
    xt = np.ascontiguousarray(np.asarray(transformer_features, np.float32)
                              .reshape(B, C, N))
    xc = np.ascontiguousarray(np.asarray(cnn_features, np.float32)
                              .reshape(B, C, N))
    Wq = np.asarray(Wq, np.float32)
    Wk = np.asarray(Wk, np.float32)
    Wv = np.asarray(Wv, np.float32)
    Wf = np.asarray(Wf, np.float32)
    bq = np.asarray(bq, np.float32)
    bk = np.asarray(bk, np.float32)
    bv = np.asarray(bv, np.float32)
    bf = np.asarray(bf, np.float32)

    wqt = np.ascontiguousarray(Wq.T / 16.0)
    wkt = np.ascontiguousarray(Wk.T)
    wvt = np.ascontiguousarray(Wv.T)
    wft = np.ascontiguousarray(Wf.T).copy()
    wft[C:] /= 256.0
    bq_s = bq / 16.0
    bf2 = bf + Wf[:, C:] @ bv

    maps = []
    for c in range(NCORES):
        b, h = divmod(c, 2)
        maps.append(dict(
            xq=np.ascontiguousarray(xt[b][:, h * QH:(h + 1) * QH]),
            xc=xc[b],
            wqt=wqt, wkt=wkt, wvt=wvt, wft=wft,
            bq=bq_s, bk=bk, bf=bf2,
        ))
    return maps


def _run(inputs, trace=False):
    from concourse.bass_utils import run_bass_kernel_spmd
    nc = _get_nc()
    maps = _in_maps(**inputs)
    return run_bass_kernel_spmd(nc, maps, list(range(NCORES)), trace=trace)


def kernel(**inputs) -> np.ndarray:
    res = _run(inputs).results
    out = np.empty((B, C, N), np.float32)
    for c in range(NCORES):
        b, h = divmod(c, 2)
        out[b][:, h * QH:(h + 1) * QH] = res[c]["out"]
    return out.reshape(B, C, H, W)
